# revision 1
# baseline (speedup 1.0000x reference)
"""Trainium2 Bass kernel for nn_Encoder_78176994721982 (E(n)-GNN encoder), 8 cores SPMD.

Strategy:
  - Edges sorted by destination (row); nodes in 128-node windows; each core owns a
    contiguous range of windows and all edges pointing into it.
  - Node features h replicated in every core's SBUF as a bf16 table
    [128 part = node%128, free = (node//128)*128 features].
  - h[col] per edge: SWDGE dma_gather (SBUF source, transposed output ->
    feature-major), spread over 4 DMA queues (~2 ns/edge).
  - h[row]: never gathered; per-window Q_w = h_win @ W1_row once, then per-subchunk
    one-hot expansion Q_w^T @ M_s on the TensorEngine.
  - segment_sum: one-hot matmul ef2^T @ M_e accumulated in PSUM per window.
  - ELU = max(z,0) + exp(min(z,0)) - 1 via ACT Relu/Exp + DVE max/add; the "-1"s
    are folded into adjusted biases and a degree-correction rank-1 matmul.
  - Node MLP / embedding / head data-parallel on node shards (h state f32).
  - One DRAM AllGather between the two layers re-replicates h.
"""

import numpy as np
import jax
import jax.numpy as jnp

import concourse.bass as bass
import concourse.mybir as mybir
import concourse.tile as tile
from concourse.tile import add_dep_helper
import concourse.bacc as bacc
from concourse.bass_utils import run_bass_kernel_spmd

P = 128
N_CORES = 8
HID = 128
LAT = 64
IN_NODE = 11
NL = 2
TAB_SPLIT = 32768
GQ = 4
GOP = 512

f32 = mybir.dt.float32
bf16 = mybir.dt.bfloat16
i16 = mybir.dt.int16
AF = mybir.ActivationFunctionType
OP = mybir.AluOpType

_compile_cache = {}


def _bf(x):
    return np.asarray(jnp.asarray(np.asarray(x), dtype=jnp.bfloat16))


def _wrap16(idx_vals):
    L = len(idx_vals)
    ops = np.asarray(idx_vals, dtype=np.int16).reshape(L // 16, 16).T
    return np.tile(ops, (8, 1))


class Plan:
    pass


def _host_prep(inputs):
    pl = Plan()
    edges = np.asarray(inputs["edges"])
    row = edges[0].astype(np.int64)
    col = edges[1].astype(np.int64)
    N = int(inputs["n_nodes"])
    NW_real = (N + P - 1) // P
    WPC = (NW_real + N_CORES - 1) // N_CORES
    NW = WPC * N_CORES
    NS = WPC * P
    NTAB = NW * P
    pl.N, pl.NW, pl.WPC, pl.NS, pl.NTAB = N, NW, WPC, NS, NTAB

    pl.split = TAB_SPLIT if NTAB > TAB_SPLIT else NTAB // 2

    x = np.asarray(inputs["x"], dtype=np.float32)
    dd = x[row] - x[col]
    radial = (dd * dd).sum(1)
    attr = np.asarray(inputs["edge_attr"], dtype=np.float32)

    order = np.argsort(row, kind="stable")
    row_s, col_s = row[order], col[order]
    win_of = row_s // P
    lo_mask = col_s < pl.split
    cnt_lo = np.zeros(NW, np.int64)
    cnt_hi = np.zeros(NW, np.int64)
    np.add.at(cnt_lo, win_of[lo_mask], 1)
    np.add.at(cnt_hi, win_of[~lo_mask], 1)
    SL = np.zeros(WPC, np.int64)
    SH = np.zeros(WPC, np.int64)
    for w in range(NW):
        SL[w % WPC] = max(SL[w % WPC], -(-cnt_lo[w] // P))
        SH[w % WPC] = max(SH[w % WPC], -(-cnt_hi[w] // P))
    SL = np.maximum(SL, 1)
    SH = np.maximum(SH, 1)
    pl.SL, pl.SH = SL, SH
    pl.TS = int((SL + SH).sum())
    pl.EP = pl.TS * P

    pl.gops = []
    for wl in range(WPC):
        for half, S in (("lo", int(SL[wl])), ("hi", int(SH[wl]))):
            n = S * P
            off = 0
            while off < n:
                L = min(GOP, n - off)
                pl.gops.append((wl, half, off, L))
                off += L
    pl.idx_cols = sum(L // 16 for (_, _, _, L) in pl.gops)

    start = np.zeros(NW + 1, np.int64)
    np.add.at(start[1:], win_of, 1)
    start = np.cumsum(start)

    seg_all = np.full((N_CORES, P, pl.TS), -1.0, np.float32)
    ra_all = np.zeros((N_CORES, 8, pl.EP), np.float32)
    idx_all = np.zeros((N_CORES, 128, pl.idx_cols), np.int16)
    deg_all = np.zeros((N_CORES, 1, NS), np.float32)
    deg_glob = np.bincount(row_s, minlength=NTAB).astype(np.float32)
    subbase = np.cumsum(np.concatenate([[0], (SL + SH)[:-1]])).astype(np.int64)

    for k in range(N_CORES):
        deg_all[k, 0, :] = deg_glob[k * NS:(k + 1) * NS]
        icol = 0
        for (wl, half, off, L) in pl.gops:
            w = k * WPC + wl
            sel = slice(start[w], start[w + 1])
            cw = col_s[sel]
            rw = row_s[sel]
            m = (cw < pl.split) if half == "lo" else (cw >= pl.split)
            cols_h = cw[m]
            rows_h = rw[m]
            eids_h = order[sel][m]
            base = 0 if half == "lo" else pl.split
            take = np.arange(off, off + L)
            valid = take < len(cols_h)
            idx_vals = np.zeros(L, np.int64)
            idx_vals[valid] = cols_h[take[valid]] - base
            idx_all[k, :, icol:icol + L // 16] = _wrap16(idx_vals)
            tcol0 = int(subbase[wl]) + (0 if half == "lo" else int(SL[wl])) + off // P
            for j in range(L // P):
                tcol = tcol0 + j
                vv = valid[j * P:(j + 1) * P]
                tk = take[j * P:(j + 1) * P]
                segv = np.full(P, -1.0, np.float32)
                segv[vv] = (rows_h[tk[vv]] - w * P).astype(np.float32)
                seg_all[k, :, tcol] = segv
                rr = np.zeros((8, P), np.float32)
                ee = eids_h[tk[vv]]
                rr[0, vv] = radial[ee]
                rr[1:5, vv] = attr[ee].T
                rr[5, vv] = 1.0
                ra_all[k, :, tcol * P:(tcol + 1) * P] = rr
            icol += L // 16

    pl.seg = seg_all.astype(np.float32)
    pl.ra = _bf(ra_all)
    pl.idx = idx_all
    pl.deg = deg_all

    h0 = np.asarray(inputs["h0"], dtype=np.float32)
    h0T = np.zeros((16, NTAB), np.float32)
    h0T[:IN_NODE, :N] = h0.T
    pl.h0T = _bf(h0T)
    pl.h0T_own = np.stack([h0T[:, k * NS:(k + 1) * NS] for k in range(N_CORES)]).astype(np.float32)

    label = np.asarray(inputs["label"], dtype=np.float32)
    lb = np.zeros((8, NTAB), np.float32)
    lb[:7, :N] = label.T
    lb[7] = 1.0
    pl.labelT = np.stack([lb[:, k * NS:(k + 1) * NS] for k in range(N_CORES)])
    eps = np.asarray(inputs["eps"], dtype=np.float32)
    ep = np.zeros((NTAB, LAT), np.float32)
    ep[:N] = eps
    pl.epsT = np.stack([np.ascontiguousarray(ep[k * NS:(k + 1) * NS].T) for k in range(N_CORES)])

    emb_w = np.zeros((16, HID), np.float32)
    emb_w[:IN_NODE] = np.asarray(inputs["emb_w"], np.float32)
    pl.emb_w = _bf(emb_w)
    pl.emb_w32 = emb_w
    pl.emb_b = np.asarray(inputs["emb_b"], np.float32).reshape(HID, 1)
    pl.emb_b_bc = np.tile(np.asarray(inputs["emb_b"], np.float32).reshape(1, HID), (P, 1))

    ew1 = np.asarray(inputs["edge_w1"], np.float32)
    eb1 = np.asarray(inputs["edge_b1"], np.float32)
    ew2 = np.asarray(inputs["edge_w2"], np.float32)
    eb2 = np.asarray(inputs["edge_b2"], np.float32)
    pl.w1r = [_bf(ew1[l, :HID]) for l in range(NL)]
    pl.w1c = [_bf(ew1[l, HID:2 * HID]) for l in range(NL)]
    w1ra = []
    for l in range(NL):
        m = np.zeros((8, HID), np.float32)
        m[0] = ew1[l, 2 * HID]
        m[1:5] = ew1[l, 2 * HID + 1:2 * HID + 5].reshape(4, HID)
        m[5] = eb1[l]
        w1ra.append(_bf(m))
    pl.w1ra = w1ra
    pl.w2 = [_bf(ew2[l]) for l in range(NL)]
    pl.b2adj = [_bf((eb2[l] - ew2[l].sum(0)).reshape(1, HID)) for l in range(NL)]

    nw1 = np.asarray(inputs["node_w1"], np.float32)
    nb1 = np.asarray(inputs["node_b1"], np.float32)
    nw2 = np.asarray(inputs["node_w2"], np.float32)
    nb2 = np.asarray(inputs["node_b2"], np.float32)
    pl.nw1ac = [(nw1[l, :HID] + nw1[l, 2 * HID:]).astype(np.float32) for l in range(NL)]
    pl.nw1b = [_bf(nw1[l, HID:2 * HID]) for l in range(NL)]
    pl.nw1deg = [(-nw1[l, HID:2 * HID].sum(0)).reshape(1, HID).astype(np.float32) for l in range(NL)]
    pl.nb1 = [nb1[l].reshape(HID, 1).astype(np.float32) for l in range(NL)]
    pl.nb1p1 = [(nb1[l] + 1.0).reshape(HID, 1).astype(np.float32) for l in range(NL)]
    pl.nw2 = [nw2[l].astype(np.float32) for l in range(NL)]
    pl.nb2adj = [(nb2[l] - nw2[l].sum(0)).reshape(HID, 1).astype(np.float32) for l in range(NL)]

    muw = np.asarray(inputs["mu_w"], np.float32)
    varw = np.asarray(inputs["var_w"], np.float32)
    pl.muw1 = muw[:HID].astype(np.float32)
    mw2 = np.zeros((8, LAT), np.float32)
    mw2[:7] = muw[HID:]
    mw2[7] = np.asarray(inputs["mu_b"], np.float32)
    pl.muw2 = mw2
    pl.varw1 = varw[:HID].astype(np.float32)
    vw2 = np.zeros((8, LAT), np.float32)
    vw2[:7] = varw[HID:]
    vw2[7] = np.asarray(inputs["var_b"], np.float32)
    pl.varw2 = vw2

    pl.iota = _bf(np.tile(np.arange(P, dtype=np.float32)[None, :], (P, 1)))
    pl.ident = _bf(np.eye(P, dtype=np.float32))
    pl.ident32 = np.eye(P, dtype=np.float32)
    pl.ones_row = _bf(np.ones((1, P), np.float32))
    return pl


def build_nc(pl, debug_taps=False, no_collective=False):
    WPC, TS, NS, NTAB = pl.WPC, pl.TS, pl.NS, pl.NTAB
    SL, SH = pl.SL, pl.SH
    SLmax, SHmax = int(SL.max()), int(SH.max())
    nc = bacc.Bacc("TRN2", target_bir_lowering=False, debug=False,
                   num_devices=N_CORES, num_swdge_queues=GQ)

    def din(name, shape, dt):
        return nc.dram_tensor(name, list(shape), dt, kind="ExternalInput").ap()

    t_idx = din("idx", [128, pl.idx_cols], i16)
    t_seg = din("seg", [P, TS], f32)
    t_ra = din("ra", [8, pl.EP], bf16)
    t_deg = din("deg", [1, NS], f32)
    t_h0T = din("h0T", [16, NTAB], bf16)
    t_h0To = din("h0T_own", [16, NS], f32)
    t_lab = din("labelT", [8, NS], f32)
    t_eps = din("epsT", [LAT, NS], f32)
    t_embw = din("emb_w", [16, HID], bf16)
    t_embw32 = din("emb_w32", [16, HID], f32)
    t_embb = din("emb_b", [HID, 1], f32)
    t_embb_bc = din("emb_b_bc", [P, HID], f32)
    t_w1r = [din(f"w1r{l}", [HID, HID], bf16) for l in range(NL)]
    t_w1c = [din(f"w1c{l}", [HID, HID], bf16) for l in range(NL)]
    t_w1ra = [din(f"w1ra{l}", [8, HID], bf16) for l in range(NL)]
    t_w2 = [din(f"w2{l}", [HID, HID], bf16) for l in range(NL)]
    t_b2adj = [din(f"b2adj{l}", [1, HID], bf16) for l in range(NL)]
    t_nw1ac = [din(f"nw1ac{l}", [HID, HID], f32) for l in range(NL)]
    t_nw1b = [din(f"nw1b{l}", [HID, HID], bf16) for l in range(NL)]
    t_nw1deg = [din(f"nw1deg{l}", [1, HID], f32) for l in range(NL)]
    t_nb1 = [din(f"nb1{l}", [HID, 1], f32) for l in range(NL)]
    t_nb1p1 = [din(f"nb1p1{l}", [HID, 1], f32) for l in range(NL)]
    t_nw2 = [din(f"nw2{l}", [HID, HID], f32) for l in range(NL)]
    t_nb2adj = [din(f"nb2adj{l}", [HID, 1], f32) for l in range(NL)]
    t_muw1 = din("muw1", [HID, LAT], f32)
    t_muw2 = din("muw2", [8, LAT], f32)
    t_varw1 = din("varw1", [HID, LAT], f32)
    t_varw2 = din("varw2", [8, LAT], f32)
    t_iota = din("iota", [P, P], bf16)
    t_ident = din("ident", [P, P], bf16)
    t_ident32 = din("ident32", [P, P], f32)
    t_ones = din("ones_row", [1, P], bf16)
    t_z = nc.dram_tensor("z", [NS, LAT], f32, kind="ExternalOutput").ap()

    if debug_taps:
        t_dbg_tab0 = nc.dram_tensor("dbg_tab0", [P, NTAB], bf16, kind="ExternalOutput").ap()
        t_dbg_agg0 = nc.dram_tensor("dbg_agg0", [P, NS], f32, kind="ExternalOutput").ap()
        t_dbg_h1 = nc.dram_tensor("dbg_h1", [P, NS], f32, kind="ExternalOutput").ap()
    cc_in = nc.dram_tensor("cc_in", [P, NS], bf16).ap()
    cc_out = nc.dram_tensor("cc_out", [N_CORES, P, NS], bf16, addr_space="Shared").ap()

    # node-dimension chunks (256 wide + remainder)
    chunks = []
    off = 0
    while off < NS:
        w = min(256, NS - off)
        chunks.append((off, w))
        off += w

    with tile.TileContext(nc) as tc:
        with tc.tile_pool(name="tabs", bufs=1) as tabs, \
             tc.tile_pool(name="const", bufs=1) as cpool, \
             tc.tile_pool(name="glo", bufs=2) as gpool, \
             tc.tile_pool(name="work", bufs=2) as wp, \
             tc.tile_pool(name="ework", bufs=3) as ew, \
             tc.tile_pool(name="rapool", bufs=1) as rap, \
             tc.tile_pool(name="pmm", bufs=1, space="PSUM") as pmm, \
             tc.tile_pool(name="pt32", bufs=1, space="PSUM") as pt32, \
             tc.tile_pool(name="ptb", bufs=2, space="PSUM") as ptb, \
             tc.tile_pool(name="pz", bufs=2, space="PSUM") as pz, \
             tc.tile_pool(name="pe2", bufs=1, space="PSUM") as pe2, \
             tc.tile_pool(name="pagg", bufs=1, space="PSUM") as pagg:

            tab = tabs.tile([P, NTAB + 16], bf16)
            hT = tabs.tile([P, NS], f32)
            aggT = tabs.tile([P, NS], bf16)
            idx_sb = tabs.tile([128, pl.idx_cols], i16)
            seg_sb = tabs.tile([P, TS], f32)

            _cseq = [0]

            def cload(shape, dt, src):
                _cseq[0] += 1
                t = cpool.tile(shape, dt, tag=f"c{_cseq[0]}")
                nc.sync.dma_start(out=t[:], in_=src[:])
                return t

            c_iota = cload([P, P], bf16, t_iota)
            c_ident = cload([P, P], bf16, t_ident)
            c_ident32 = cload([P, P], f32, t_ident32)
            c_ones = cload([1, P], bf16, t_ones)
            c_embw = cload([16, HID], bf16, t_embw)
            c_embw32 = cload([16, HID], f32, t_embw32)
            c_embb = cload([HID, 1], f32, t_embb)
            c_embb_bc = cload([P, HID], f32, t_embb_bc)
            c_w1r = [cload([HID, HID], bf16, t_w1r[l]) for l in range(NL)]
            c_w1c = [cload([HID, HID], bf16, t_w1c[l]) for l in range(NL)]
            c_w1ra = [cload([8, HID], bf16, t_w1ra[l]) for l in range(NL)]
            c_w2 = [cload([HID, HID], bf16, t_w2[l]) for l in range(NL)]
            c_b2 = [cload([1, HID], bf16, t_b2adj[l]) for l in range(NL)]
            c_nw1ac = [cload([HID, HID], f32, t_nw1ac[l]) for l in range(NL)]
            c_nw1b = [cload([HID, HID], bf16, t_nw1b[l]) for l in range(NL)]
            c_nw1dg = [cload([1, HID], f32, t_nw1deg[l]) for l in range(NL)]
            c_nb1 = [cload([HID, 1], f32, t_nb1[l]) for l in range(NL)]
            c_nb1p1 = [cload([HID, 1], f32, t_nb1p1[l]) for l in range(NL)]
            c_nw2 = [cload([HID, HID], f32, t_nw2[l]) for l in range(NL)]
            c_nb2 = [cload([HID, 1], f32, t_nb2adj[l]) for l in range(NL)]
            c_muw1 = cload([HID, LAT], f32, t_muw1)
            c_muw2 = cload([8, LAT], f32, t_muw2)
            c_varw1 = cload([HID, LAT], f32, t_varw1)
            c_varw2 = cload([8, LAT], f32, t_varw2)

            nc.sync.dma_start(out=idx_sb[:], in_=t_idx[:])
            nc.sync.dma_start(out=seg_sb[:], in_=t_seg[:])

            gsems = [nc.alloc_semaphore(f"gsem{q}") for q in range(GQ)]
            gcount = [0]
            qcount = [0] * GQ

            gop_icol = []
            icol = 0
            for (_, _, _, L) in pl.gops:
                gop_icol.append(icol)
                icol += L // 16

            def issue_gathers(wl):
                g_lo = gpool.tile([P, 1, SLmax * P + 16], bf16, tag="glo")
                g_hi = gpool.tile([P, 1, SHmax * P + 16], bf16, tag="ghi")
                gins = []
                for gi, (wl2, half, off, L) in enumerate(pl.gops):
                    if wl2 != wl:
                        continue
                    g = g_lo if half == "lo" else g_hi
                    src = tab[:, 0:pl.split] if half == "lo" else tab[:, pl.split:NTAB]
                    q = gcount[0] % GQ
                    with tc.tile_critical():
                        gg = nc.gpsimd.dma_gather(
                            out_ap=g[:, :, off:off + L],
                            in_ap=src,
                            idxs_ap=idx_sb[:, gop_icol[gi]:gop_icol[gi] + L // 16],
                            num_idxs=L, num_idxs_reg=L, elem_size=HID,
                            transpose=True,
                            sbuf_tokens_per_rank=128,
                            sbuf_free_dim_per_rank=256,
                            queue_num=q,
                        )
                        gg.then_inc(gsems[q], 16)
                    gins.append(gg.ins)
                    gcount[0] += 1
                    qcount[q] += 1
                return g_lo, g_hi, tuple(qcount), gins

            def wait_gathers(g_lo, g_hi, cums, gins):
                with tc.tile_critical():
                    wi = None
                    for q in range(GQ):
                        if cums[q]:
                            wi = nc.gpsimd.wait_ge(gsems[q], 16 * cums[q])
                            for gi_ins in gins:
                                add_dep_helper(wi.ins, gi_ins, sync=False,
                                               reason="gather wait ordering")
                    nc.gpsimd.memset(g_lo[:, :, SLmax * P:], 0)
                    nc.gpsimd.memset(g_hi[:, :, SHmax * P:], 0)

            # ---------- embedding: replicated node-major table ----------
            for c in range(NTAB // 512):
                h0t = wp.tile([16, 512], bf16, tag="h0t")
                nc.sync.dma_start(out=h0t[:], in_=t_h0T[:, c * 512:(c + 1) * 512])
                for j in range(4):
                    ps = pmm.tile([P, P], f32, tag="pmm")
                    nc.tensor.matmul(out=ps[:], lhsT=h0t[:, j * P:(j + 1) * P],
                                     rhs=c_embw[:], start=True, stop=True)
                    w = c * 4 + j
                    nc.vector.tensor_tensor(out=tab[:, w * P:(w + 1) * P], in0=ps[:],
                                            in1=c_embb_bc[:], op=OP.add)
            # own-shard hT (f32, feature-major)
            for (coff, cw) in chunks:
                h0o = wp.tile([16, 256], f32, tag="h0o")
                nc.sync.dma_start(out=h0o[:, :cw], in_=t_h0To[:, coff:coff + cw])
                ps = pmm.tile([P, 256], f32, tag="pmm")
                nc.tensor.matmul(out=ps[:, :cw], lhsT=c_embw32[:], rhs=h0o[:, :cw],
                                 start=True, stop=True)
                nc.scalar.activation(out=hT[:, coff:coff + cw], in_=ps[:, :cw],
                                     func=AF.Identity, bias=c_embb[:])

            if debug_taps:
                nc.sync.dma_start(out=t_dbg_tab0[:], in_=tab[:, :NTAB])
            # ---------- message-passing layers ----------
            subbase = np.cumsum(np.concatenate([[0], (SL + SH)[:-1]])).astype(np.int64)
            for l in range(NL):
                pending = [issue_gathers(0)]
                if WPC > 1:
                    pending.append(issue_gathers(1))
                for wl in range(WPC):
                    g_lo, g_hi, cums, gins = pending.pop(0)
                    wait_gathers(g_lo, g_hi, cums, gins)
                    if wl + 2 < WPC:
                        pending.append(issue_gathers(wl + 2))

                    # per-window: Q_w = h_win @ W1r (h from the local f32 shard)
                    hTw = wp.tile([P, P], bf16, tag="hTw")
                    nc.vector.tensor_copy(out=hTw[:], in_=hT[:, wl * P:(wl + 1) * P])
                    qps = pt32.tile([P, P], f32, tag="pt32")
                    nc.tensor.matmul(out=qps[:], lhsT=hTw[:], rhs=c_w1r[l][:],
                                     start=True, stop=True)
                    qw = wp.tile([P, P], bf16, tag="qw")
                    nc.scalar.copy(out=qw[:], in_=qps[:])
                    Sw = int(SL[wl] + SH[wl])
                    sub_t = int(subbase[wl])
                    ra = rap.tile([8, (SLmax + SHmax) * P], bf16, tag="ra")
                    nc.sync.dma_start(out=ra[:, :Sw * P],
                                      in_=t_ra[:, sub_t * P:(sub_t + Sw) * P])
                    p_agg = pagg.tile([P, P], f32, tag="pagg")

                    for s in range(Sw):
                        half_lo = s < int(SL[wl])
                        goff = s * P if half_lo else (s - int(SL[wl])) * P
                        g = g_lo if half_lo else g_hi
                        tcol = sub_t + s
                        me = ew.tile([P, P], bf16, tag="me")
                        nc.vector.tensor_scalar(
                            out=me[:], in0=c_iota[:],
                            scalar1=seg_sb[:, tcol:tcol + 1],
                            scalar2=None, op0=OP.is_equal)
                        msp = ptb.tile([P, P], bf16, tag="ptb")
                        nc.tensor.transpose(out=msp[:], in_=me[:], identity=c_ident[:])
                        ms = ew.tile([P, P], bf16, tag="ms")
                        nc.scalar.copy(out=ms[:], in_=msp[:])

                        p_z = pz.tile([P, P], f32, tag="pz")
                        nc.tensor.matmul(out=p_z[:], lhsT=qw[:], rhs=ms[:],
                                         start=True, stop=False)
                        nc.tensor.matmul(out=p_z[:], lhsT=c_w1c[l][:],
                                         rhs=g[:, 0, goff:goff + P],
                                         start=False, stop=False)
                        nc.tensor.matmul(out=p_z[:], lhsT=c_w1ra[l][:],
                                         rhs=ra[:, s * P:(s + 1) * P],
                                         start=False, stop=True)
                        # elu(z)+1 == max(z,0) + exp(min(z,0))
                        m1 = ew.tile([P, P], bf16, tag="mm1")
                        nc.vector.tensor_scalar(out=m1[:], in0=p_z[:], scalar1=0.0,
                                                scalar2=None, op0=OP.min)
                        e1 = ew.tile([P, P], bf16, tag="ee")
                        nc.scalar.activation(out=e1[:], in_=m1[:], func=AF.Exp)
                        r1 = ew.tile([P, P], bf16, tag="rr")
                        nc.vector.tensor_scalar(out=r1[:], in0=p_z[:], scalar1=0.0,
                                                scalar2=None, op0=OP.max)
                        ef1 = ew.tile([P, P], bf16, tag="ef")
                        nc.vector.tensor_tensor(out=ef1[:], in0=r1[:], in1=e1[:],
                                                op=OP.add)
                        p_e2 = pe2.tile([P, P], f32, tag="pe2")
                        nc.tensor.matmul(out=p_e2[:], lhsT=ef1[:], rhs=c_w2[l][:],
                                         start=True, stop=False)
                        nc.tensor.matmul(out=p_e2[:], lhsT=c_ones[:], rhs=c_b2[l][:],
                                         start=False, stop=True)
                        t2 = ew.tile([P, P], f32, tag="tt")
                        nc.scalar.activation(out=t2[:], in_=p_e2[:], func=AF.Relu,
                                             scale=-1.0)
                        e2 = ew.tile([P, P], f32, tag="ee")
                        nc.scalar.activation(out=e2[:], in_=t2[:], func=AF.Exp,
                                             scale=-1.0)
                        r2 = ew.tile([P, P], f32, tag="rr")
                        nc.vector.tensor_scalar(out=r2[:], in0=p_e2[:], scalar1=0.0,
                                                scalar2=None, op0=OP.max)
                        ef2 = ew.tile([P, P], bf16, tag="ef")
                        nc.vector.tensor_tensor(out=ef2[:], in0=r2[:], in1=e2[:],
                                                op=OP.add)
                        nc.tensor.matmul(out=p_agg[:], lhsT=ef2[:], rhs=me[:],
                                         start=(s == 0), stop=(s == Sw - 1))
                    nc.vector.tensor_copy(out=aggT[:, wl * P:(wl + 1) * P],
                                          in_=p_agg[:])

                if debug_taps and l == 0:
                    dagg = wp.tile([P, 256], f32, tag="dagg")
                    for (coff, cw) in chunks:
                        nc.vector.tensor_copy(out=dagg[:, :cw], in_=aggT[:, coff:coff + cw])
                        nc.sync.dma_start(out=t_dbg_agg0[:, coff:coff + cw], in_=dagg[:, :cw])
                # ---------- node MLP ----------
                for (coff, cw) in chunks:
                    sl = slice(coff, coff + cw)
                    p_nf = pmm.tile([P, 256], f32, tag="pmm")
                    nc.tensor.matmul(out=p_nf[:, :cw], lhsT=c_nw1ac[l][:],
                                     rhs=hT[:, sl], start=True, stop=False)
                    nc.tensor.matmul(out=p_nf[:, :cw], lhsT=c_nw1b[l][:],
                                     rhs=aggT[:, sl], start=False, stop=False)
                    degc = wp.tile([1, 256], f32, tag="degc")
                    nc.sync.dma_start(out=degc[:, :cw], in_=t_deg[:, sl])
                    nc.tensor.matmul(out=p_nf[:, :cw], lhsT=c_nw1dg[l][:],
                                     rhs=degc[:, :cw], start=False, stop=True)
                    mn = wp.tile([P, 256], f32, tag="tn")
                    nc.vector.tensor_scalar(out=mn[:, :cw], in0=p_nf[:, :cw],
                                            scalar1=c_nb1[l][:], scalar2=0.0,
                                            op0=OP.add, op1=OP.min)
                    en = wp.tile([P, 256], f32, tag="en")
                    nc.scalar.activation(out=en[:, :cw], in_=mn[:, :cw], func=AF.Exp)
                    rn = wp.tile([P, 256], f32, tag="rn")
                    nc.vector.tensor_scalar(out=rn[:, :cw], in0=p_nf[:, :cw],
                                            scalar1=c_nb1[l][:], scalar2=0.0,
                                            op0=OP.add, op1=OP.max)
                    nf1 = wp.tile([P, 256], f32, tag="nf1")
                    nc.vector.tensor_tensor(out=nf1[:, :cw], in0=rn[:, :cw],
                                            in1=en[:, :cw], op=OP.add)
                    p_h = pmm.tile([P, 256], f32, tag="pmm")
                    nc.tensor.matmul(out=p_h[:, :cw], lhsT=c_nw2[l][:],
                                     rhs=nf1[:, :cw], start=True, stop=True)
                    nc.scalar.activation(out=hT[:, sl], in_=p_h[:, :cw],
                                         func=AF.Identity, bias=c_nb2[l][:])

                if debug_taps and l == 0:
                    nc.sync.dma_start(out=t_dbg_h1[:], in_=hT[:])
                # ---------- layer boundary: re-replicate h ----------
                if l == 0:
                    for wl in range(WPC):
                        tp = pt32.tile([P, P], f32, tag="pt32")
                        nc.tensor.transpose(out=tp[:],
                                            in_=hT[:, wl * P:(wl + 1) * P],
                                            identity=c_ident32[:])
                        hb = wp.tile([P, P], bf16, tag="hb")
                        nc.vector.tensor_copy(out=hb[:], in_=tp[:])
                        nc.sync.dma_start(out=cc_in[:, wl * P:(wl + 1) * P],
                                          in_=hb[:])
                    if not no_collective:
                        nc.gpsimd.collective_compute(
                            "AllGather", OP.bypass,
                            replica_groups=[list(range(N_CORES))],
                            ins=[cc_in[:]], outs=[cc_out[:]],
                        )
                    nc.sync.dma_start(
                        out=tab[:, :NTAB].rearrange("p (k c) -> p k c", k=N_CORES),
                        in_=cc_out.rearrange("k p c -> p k c"),
                    )

            # ---------- head ----------
            for wl in range(WPC):
                sl = slice(wl * P, (wl + 1) * P)
                lab = wp.tile([8, P], f32, tag="lab")
                nc.sync.dma_start(out=lab[:], in_=t_lab[:, sl])
                epst = wp.tile([LAT, P], f32, tag="epst")
                nc.sync.dma_start(out=epst[:], in_=t_eps[:, sl])
                p_mu = pz.tile([LAT, P], f32, tag="pz")
                nc.tensor.matmul(out=p_mu[:], lhsT=c_muw1[:], rhs=hT[:, sl],
                                 start=True, stop=False)
                nc.tensor.matmul(out=p_mu[:], lhsT=c_muw2[:], rhs=lab[:],
                                 start=False, stop=True)
                p_lv = pe2.tile([LAT, P], f32, tag="pe2")
                nc.tensor.matmul(out=p_lv[:], lhsT=c_varw1[:], rhs=hT[:, sl],
                                 start=True, stop=False)
                nc.tensor.matmul(out=p_lv[:], lhsT=c_varw2[:], rhs=lab[:],
                                 start=False, stop=True)
                std = wp.tile([LAT, P], f32, tag="std")
                nc.scalar.activation(out=std[:], in_=p_lv[:], func=AF.Exp, scale=0.5)
                m1 = wp.tile([LAT, P], f32, tag="m1")
                nc.vector.tensor_tensor(out=m1[:], in0=epst[:], in1=std[:], op=OP.mult)
                m2 = wp.tile([LAT, P], f32, tag="m2")
                nc.vector.tensor_scalar(out=m2[:], in0=m1[:], scalar1=0.01,
                                        scalar2=None, op0=OP.mult)
                zT = wp.tile([LAT, P], f32, tag="zT")
                nc.vector.tensor_tensor(out=zT[:], in0=p_mu[:], in1=m2[:], op=OP.add)
                ztp = pt32.tile([P, LAT], f32, tag="pt32")
                nc.tensor.transpose(out=ztp[:], in_=zT[:], identity=c_ident32[:LAT, :LAT])
                zsb = wp.tile([P, LAT], f32, tag="zsb")
                nc.vector.tensor_copy(out=zsb[:], in_=ztp[:])
                nc.sync.dma_start(out=t_z[wl * P:(wl + 1) * P, :], in_=zsb[:])
    nc.compile()
    return nc


def _in_maps(pl):
    maps = []
    for k in range(N_CORES):
        m = {
            "idx": pl.idx[k], "seg": pl.seg[k], "ra": pl.ra[k], "deg": pl.deg[k],
            "h0T": pl.h0T, "h0T_own": pl.h0T_own[k], "labelT": pl.labelT[k],
            "epsT": pl.epsT[k],
            "emb_w": pl.emb_w, "emb_w32": pl.emb_w32, "emb_b": pl.emb_b,
            "emb_b_bc": pl.emb_b_bc,
            "muw1": pl.muw1, "muw2": pl.muw2, "varw1": pl.varw1, "varw2": pl.varw2,
            "iota": pl.iota, "ident": pl.ident, "ident32": pl.ident32,
            "ones_row": pl.ones_row,
        }
        for l in range(NL):
            m[f"w1r{l}"] = pl.w1r[l]
            m[f"w1c{l}"] = pl.w1c[l]
            m[f"w1ra{l}"] = pl.w1ra[l]
            m[f"w2{l}"] = pl.w2[l]
            m[f"b2adj{l}"] = pl.b2adj[l]
            m[f"nw1ac{l}"] = pl.nw1ac[l]
            m[f"nw1b{l}"] = pl.nw1b[l]
            m[f"nw1deg{l}"] = pl.nw1deg[l]
            m[f"nb1{l}"] = pl.nb1[l]
            m[f"nb1p1{l}"] = pl.nb1p1[l]
            m[f"nw2{l}"] = pl.nw2[l]
            m[f"nb2adj{l}"] = pl.nb2adj[l]
        maps.append(m)
    return maps


def _prep_and_build(inputs):
    pl = _host_prep(inputs)
    key = (pl.TS, tuple(pl.SL), tuple(pl.SH), pl.idx_cols, pl.NTAB, pl.split)
    if key not in _compile_cache:
        _compile_cache[key] = build_nc(pl)
    return pl, _compile_cache[key]


def kernel(**inputs):
    pl, nc = _prep_and_build(inputs)
    maps = _in_maps(pl)
    res = run_bass_kernel_spmd(nc, maps, list(range(N_CORES)))
    z = np.concatenate([res.results[k]["z"] for k in range(N_CORES)], axis=0)
    return z[:pl.N].astype(np.float32)



# revision 2
# speedup vs baseline: 56.5885x; 56.5885x over previous
"""Trainium2 Bass kernel for nn_Encoder_78176994721982 (E(n)-GNN encoder), 8 cores SPMD.

v2 design (vs baseline):
  - 512-edge macro-blocks: all elementwise ELU work in [128, 512] tiles
    (4x fewer DVE/ACT instructions, fixed overheads amortized).
  - One-hot scatter (me) / row-expansion (ms) matrices precomputed on HOST
    and DMA-streamed per window -- no per-subtile is_equal / transpose / copy.
  - ELU split across engines: stage-1 DVE-heavy (copy+min+max+add on DVE,
    exp on ACT), stage-2 ACT-heavy (copy+exp on ACT, min+max+add on DVE).
  - Node MLP + head fused into the window loop per window-pair (256 nodes),
    f32r matmuls (1 cycle/row at N>=256); aggT buffer eliminated.
  - Gather idx streamed per window; h0T carries a 1.0 row so the embedding
    bias folds into the matmul.
"""

import numpy as np
import jax
import jax.numpy as jnp
import ml_dtypes

import concourse.bass as bass
import concourse.mybir as mybir
import concourse.tile as tile
from concourse.tile import add_dep_helper
import concourse.bacc as bacc
from concourse.bass_utils import run_bass_kernel_spmd

P = 128
N_CORES = 8
HID = 128
LAT = 64
IN_NODE = 11
NL = 2
TAB_SPLIT = 32768
GOP = 896
GQ = 4

f32 = mybir.dt.float32
f32r = mybir.dt.float32r
f8 = mybir.dt.float8e4
bf16 = mybir.dt.bfloat16
i16 = mybir.dt.int16
AF = mybir.ActivationFunctionType
OP = mybir.AluOpType

_compile_cache = {}
EDGE_ONLY = False
ABL_NO_SCATTER = False
ABL_NO_STAGE23 = False
ABL_NO_G = False
MEMS_FP8 = True


def _bf(x):
    return np.asarray(jnp.asarray(np.asarray(x), dtype=jnp.bfloat16))


def _onehot_bits(eq_bool):
    if MEMS_FP8:
        return (eq_bool.astype(np.uint8) * np.uint8(0x38)).view(ml_dtypes.float8_e4m3fn)
    return (eq_bool.astype(np.uint16) * np.uint16(0x3F80)).view(ml_dtypes.bfloat16)


def _wrap16(idx_vals):
    L = len(idx_vals)
    ops = np.asarray(idx_vals, dtype=np.int16).reshape(L // 16, 16).T
    return np.tile(ops, (8, 1))


class Plan:
    pass


def _host_prep(inputs):
    pl = Plan()
    edges = np.asarray(inputs["edges"])
    row = edges[0].astype(np.int64)
    col = edges[1].astype(np.int64)
    N = int(inputs["n_nodes"])
    NW_real = (N + P - 1) // P
    WPC = (NW_real + N_CORES - 1) // N_CORES
    NW = WPC * N_CORES
    NS = WPC * P
    NTAB = NW * P
    pl.N, pl.NW, pl.WPC, pl.NS, pl.NTAB = N, NW, WPC, NS, NTAB
    pl.split = TAB_SPLIT if NTAB > TAB_SPLIT else NTAB // 2

    x = np.asarray(inputs["x"], dtype=np.float32)
    dd = x[row] - x[col]
    radial = (dd * dd).sum(1).astype(np.float32)
    attr = np.asarray(inputs["edge_attr"], dtype=np.float32)

    order = np.argsort(row, kind="stable")
    row_s, col_s = row[order], col[order]
    win_of = row_s // P
    lo_mask = col_s < pl.split
    cnt_lo = np.zeros(NW, np.int64)
    cnt_hi = np.zeros(NW, np.int64)
    np.add.at(cnt_lo, win_of[lo_mask], 1)
    np.add.at(cnt_hi, win_of[~lo_mask], 1)
    SL = np.zeros(WPC, np.int64)
    SH = np.zeros(WPC, np.int64)
    for w in range(NW):
        SL[w % WPC] = max(SL[w % WPC], -(-cnt_lo[w] // P))
        SH[w % WPC] = max(SH[w % WPC], -(-cnt_hi[w] // P))
    SL = np.maximum(SL, 1)
    SH = np.maximum(SH, 1)
    pl.SL, pl.SH = SL, SH
    Sw = SL + SH
    pl.Sw = Sw
    pl.SwMax = int(Sw.max())
    pl.TS = int(Sw.sum())
    pl.EP = pl.TS * P
    subbase = np.cumsum(np.concatenate([[0], Sw[:-1]])).astype(np.int64)
    pl.subbase = subbase

    # gather chunk table, shared across cores: (wl, is_lo, off, L, icol_rel)
    gops = []
    for wl in range(WPC):
        for is_lo, S in ((True, int(SL[wl])), (False, int(SH[wl]))):
            n = S * P
            off = 0
            while off < n:
                L = min(GOP, n - off)
                icol_rel = (0 if is_lo else int(SL[wl]) * 8) + off // 16
                gops.append((wl, is_lo, off, L, icol_rel))
                off += L
    pl.gops = gops
    pl.idx_cols = pl.TS * 8

    start = np.zeros(NW + 1, np.int64)
    np.add.at(start[1:], win_of, 1)
    start = np.cumsum(start)

    deg_glob = np.bincount(row_s, minlength=NTAB).astype(np.float32)

    idx_all = np.zeros((N_CORES, 128, pl.idx_cols), np.int16)
    ra_all = np.zeros((N_CORES, 8, pl.EP), np.float32)
    mems_all = np.zeros((N_CORES, 128, 2 * pl.EP),
                        dtype=ml_dtypes.float8_e4m3fn if MEMS_FP8 else ml_dtypes.bfloat16)
    deg_all = np.zeros((N_CORES, 1, NS), np.float32)  # cast to bf16 below
    nrange = np.arange(P)

    for k in range(N_CORES):
        deg_all[k, 0, :] = deg_glob[k * NS:(k + 1) * NS]
        for wl in range(WPC):
            w = k * WPC + wl
            S_lo, S_hi = int(SL[wl]), int(SH[wl])
            sww = S_lo + S_hi
            sel = slice(start[w], start[w + 1])
            cw = col_s[sel]
            rw = row_s[sel]
            eid = order[sel]
            m = cw < pl.split
            # padded per-window edge arrays: [lo block | hi block]
            npad = sww * P
            seg_pad = np.full(npad, -1, np.int64)
            col_pad = np.zeros(npad, np.int64)
            eid_pad = np.zeros(npad, np.int64)
            valid = np.zeros(npad, bool)
            for is_lo, S, boff in ((True, S_lo, 0), (False, S_hi, S_lo * P)):
                mm_ = m if is_lo else ~m
                c = int(mm_.sum())
                base = 0 if is_lo else pl.split
                seg_pad[boff:boff + c] = rw[mm_] - w * P
                col_pad[boff:boff + c] = cw[mm_] - base
                eid_pad[boff:boff + c] = eid[mm_]
                valid[boff:boff + c] = True
            # gather idx
            icol0 = int(subbase[wl]) * 8
            idx_all[k, :, icol0:icol0 + sww * 8] = _wrap16(col_pad)
            # ra
            rb = int(subbase[wl]) * P
            rr = np.zeros((8, npad), np.float32)
            ev = eid_pad[valid]
            rr[0, valid] = radial[ev]
            rr[1:5, valid] = attr[ev].T
            rr[5, valid] = 1.0
            ra_all[k, :, rb:rb + npad] = rr
            # ms [node, edge] / me [edge-in-subtile, node] one-hots
            mb = 2 * rb
            ms = _onehot_bits(seg_pad[None, :] == nrange[:, None])
            seg_r = seg_pad.reshape(sww, P)
            me = _onehot_bits(
                np.transpose(seg_r[:, :, None] == nrange[None, None, :],
                             (1, 0, 2)).reshape(P, npad))
            mems_all[k, :, mb:mb + npad] = ms
            mems_all[k, :, mb + npad:mb + 2 * npad] = me

    pl.idx = idx_all
    pl.ra = _bf(ra_all)
    pl.mems = mems_all
    pl.deg = _bf(deg_all)

    h0 = np.asarray(inputs["h0"], dtype=np.float32)
    h0T = np.zeros((16, NTAB), np.float32)
    h0T[:IN_NODE, :N] = h0.T
    h0T[IN_NODE] = 1.0
    pl.h0T = _bf(h0T)
    pl.h0T_own = np.stack(
        [h0T[:, k * NS:(k + 1) * NS] for k in range(N_CORES)]).astype(np.float32)

    label = np.asarray(inputs["label"], dtype=np.float32)
    lb = np.zeros((8, NTAB), np.float32)
    lb[:7, :N] = label.T
    lb[7] = 1.0
    pl.labelT = _bf(np.stack([lb[:, k * NS:(k + 1) * NS] for k in range(N_CORES)]))
    eps = np.asarray(inputs["eps"], dtype=np.float32)
    ep = np.zeros((NTAB, LAT), np.float32)
    ep[:N] = 0.01 * eps
    pl.epsT = np.stack(
        [np.ascontiguousarray(ep[k * NS:(k + 1) * NS].T) for k in range(N_CORES)])

    emb_w = np.zeros((16, HID), np.float32)
    emb_w[:IN_NODE] = np.asarray(inputs["emb_w"], np.float32)
    emb_w[IN_NODE] = np.asarray(inputs["emb_b"], np.float32)
    pl.emb16 = _bf(emb_w)
    pl.emb16_32 = emb_w

    ew1 = np.asarray(inputs["edge_w1"], np.float32)
    eb1 = np.asarray(inputs["edge_b1"], np.float32)
    ew2 = np.asarray(inputs["edge_w2"], np.float32)
    eb2 = np.asarray(inputs["edge_b2"], np.float32)
    pl.w1r32 = [ew1[l, :HID].astype(np.float32) for l in range(NL)]
    pl.w1c = [_bf(ew1[l, HID:2 * HID]) for l in range(NL)]
    w1ra = []
    for l in range(NL):
        mat = np.zeros((8, HID), np.float32)
        mat[0] = ew1[l, 2 * HID]
        mat[1:5] = ew1[l, 2 * HID + 1:2 * HID + 5].reshape(4, HID)
        mat[5] = eb1[l]
        w1ra.append(_bf(mat))
    pl.w1ra = w1ra
    pl.w2 = [_bf(ew2[l]) for l in range(NL)]
    pl.b2adj = [_bf((eb2[l] - ew2[l].sum(0)).reshape(1, HID)) for l in range(NL)]

    nw1 = np.asarray(inputs["node_w1"], np.float32)
    nb1 = np.asarray(inputs["node_b1"], np.float32)
    nw2 = np.asarray(inputs["node_w2"], np.float32)
    nb2 = np.asarray(inputs["node_b2"], np.float32)
    pl.nw1ac = [_bf(nw1[l, :HID] + nw1[l, 2 * HID:]) for l in range(NL)]
    pl.nw1b = [_bf(nw1[l, HID:2 * HID]) for l in range(NL)]
    pl.nw1deg = [_bf((-nw1[l, HID:2 * HID].sum(0)).reshape(1, HID))
                 for l in range(NL)]
    pl.nb1 = [nb1[l].reshape(HID, 1).astype(np.float32) for l in range(NL)]
    pl.nw2 = [_bf(nw2[l]) for l in range(NL)]
    pl.nb2adj = [(nb2[l] - nw2[l].sum(0)).reshape(HID, 1).astype(np.float32)
                 for l in range(NL)]

    muw = np.asarray(inputs["mu_w"], np.float32)
    varw = np.asarray(inputs["var_w"], np.float32)
    pl.muw1 = _bf(muw[:HID])
    mw2 = np.zeros((8, LAT), np.float32)
    mw2[:7] = muw[HID:]
    mw2[7] = np.asarray(inputs["mu_b"], np.float32)
    pl.muw2 = _bf(mw2)
    pl.varw1 = _bf(varw[:HID])
    vw2 = np.zeros((8, LAT), np.float32)
    vw2[:7] = varw[HID:]
    vw2[7] = np.asarray(inputs["var_b"], np.float32)
    pl.varw2 = _bf(vw2)

    pl.ident32 = np.eye(P, dtype=np.float32)
    pl.ones_row = _bf(np.ones((1, P), np.float32))
    return pl


def build_nc(pl, no_collective=False):
    WPC, TS, NS, NTAB = pl.WPC, pl.TS, pl.NS, pl.NTAB
    SL, SH, Sw, subbase = pl.SL, pl.SH, pl.Sw, pl.subbase
    SwMax = pl.SwMax
    nc = bacc.Bacc("TRN2", target_bir_lowering=False, debug=False,
                   num_devices=N_CORES, num_swdge_queues=GQ)

    def din(name, shape, dt):
        return nc.dram_tensor(name, list(shape), dt, kind="ExternalInput").ap()

    t_idx = din("idx", [128, pl.idx_cols], i16)
    t_ra = din("ra", [8, pl.EP], bf16)
    t_mems = din("mems", [128, 2 * pl.EP], f8 if MEMS_FP8 else bf16)
    t_deg = din("deg", [1, NS], bf16)
    t_h0T = din("h0T", [16, NTAB], bf16)
    t_h0To = din("h0T_own", [16, NS], f32)
    t_lab = din("labelT", [8, NS], bf16)
    t_eps = din("epsT", [LAT, NS], f32)
    t_emb16 = din("emb16", [16, HID], bf16)
    t_emb16_32 = din("emb16_32", [16, HID], f32)
    t_w1r32 = [din(f"w1r32{l}", [HID, HID], f32) for l in range(NL)]
    t_w1c = [din(f"w1c{l}", [HID, HID], bf16) for l in range(NL)]
    t_w1ra = [din(f"w1ra{l}", [8, HID], bf16) for l in range(NL)]
    t_w2 = [din(f"w2{l}", [HID, HID], bf16) for l in range(NL)]
    t_b2adj = [din(f"b2adj{l}", [1, HID], bf16) for l in range(NL)]
    t_nw1ac = [din(f"nw1ac{l}", [HID, HID], bf16) for l in range(NL)]
    t_nw1b = [din(f"nw1b{l}", [HID, HID], bf16) for l in range(NL)]
    t_nw1deg = [din(f"nw1deg{l}", [1, HID], bf16) for l in range(NL)]
    t_nb1 = [din(f"nb1{l}", [HID, 1], f32) for l in range(NL)]
    t_nw2 = [din(f"nw2{l}", [HID, HID], bf16) for l in range(NL)]
    t_nb2adj = [din(f"nb2adj{l}", [HID, 1], f32) for l in range(NL)]
    t_muw1 = din("muw1", [HID, LAT], bf16)
    t_muw2 = din("muw2", [8, LAT], bf16)
    t_varw1 = din("varw1", [HID, LAT], bf16)
    t_varw2 = din("varw2", [8, LAT], bf16)
    t_ident32 = din("ident32", [P, P], f32)
    t_ones = din("ones_row", [1, P], bf16)
    t_z = nc.dram_tensor("z", [LAT, NS], f32, kind="ExternalOutput").ap()

    cc_in = nc.dram_tensor("cc_in", [P, NS], bf16).ap()
    cc_out = nc.dram_tensor("cc_out", [N_CORES, P, NS], bf16, addr_space="Shared").ap()

    with tile.TileContext(nc) as tc:
        with tc.tile_pool(name="tabs", bufs=1) as tabs, \
             tc.tile_pool(name="const", bufs=1) as cpool, \
             tc.tile_pool(name="gpool", bufs=2) as gpool, \
             tc.tile_pool(name="ipool", bufs=4) as ipool, \
             tc.tile_pool(name="mpool", bufs=2) as mpool, \
             tc.tile_pool(name="rapool", bufs=2) as rapool, \
             tc.tile_pool(name="wp", bufs=2) as wp, \
             tc.tile_pool(name="ew", bufs=2) as ew, \
             tc.tile_pool(name="np_", bufs=2) as npl, \
             tc.tile_pool(name="hp", bufs=2) as hp, \
             tc.tile_pool(name="pz", bufs=3, space="PSUM") as pz, \
             tc.tile_pool(name="pe2", bufs=2, space="PSUM") as pe2, \
             tc.tile_pool(name="pagg", bufs=1, space="PSUM") as pagg, \
             tc.tile_pool(name="pqs", bufs=1, space="PSUM") as pqs, \
             tc.tile_pool(name="pnf", bufs=1, space="PSUM") as pnf:

            tab = tabs.tile([P, NTAB + 16], bf16)
            hT = tabs.tile([P, NS], f32)


            _cseq = [0]

            def cload(shape, dt, src):
                _cseq[0] += 1
                t = cpool.tile(shape, dt, tag=f"c{_cseq[0]}")
                nc.sync.dma_start(out=t[:], in_=src[:])
                return t

            c_ident32 = cload([P, P], f32, t_ident32)
            c_ones = cload([1, P], bf16, t_ones)
            c_emb16 = cload([16, HID], bf16, t_emb16)
            c_emb16_32 = cload([16, HID], f32, t_emb16_32)
            c_w1r32 = [cload([HID, HID], f32, t_w1r32[l]) for l in range(NL)]
            c_w1c = [cload([HID, HID], bf16, t_w1c[l]) for l in range(NL)]
            c_w1ra = [cload([8, HID], bf16, t_w1ra[l]) for l in range(NL)]
            c_w2 = [cload([HID, HID], bf16, t_w2[l]) for l in range(NL)]
            c_b2 = [cload([1, HID], bf16, t_b2adj[l]) for l in range(NL)]
            c_nw1ac = [cload([HID, HID], bf16, t_nw1ac[l]) for l in range(NL)]
            c_nw1b = [cload([HID, HID], bf16, t_nw1b[l]) for l in range(NL)]
            c_nw1dg = [cload([1, HID], bf16, t_nw1deg[l]) for l in range(NL)]
            c_nb1 = [cload([HID, 1], f32, t_nb1[l]) for l in range(NL)]
            c_nw2 = [cload([HID, HID], bf16, t_nw2[l]) for l in range(NL)]
            c_nb2 = [cload([HID, 1], f32, t_nb2adj[l]) for l in range(NL)]
            c_muw1 = cload([HID, LAT], bf16, t_muw1)
            c_muw2 = cload([8, LAT], bf16, t_muw2)
            c_varw1 = cload([HID, LAT], bf16, t_varw1)
            c_varw2 = cload([8, LAT], bf16, t_varw2)

            gsems = [nc.alloc_semaphore(f"gsem{q}") for q in range(GQ)]
            gcount = [0]
            qcount = [0] * GQ

            def issue_gathers(wl):
                S_lo = int(SL[wl])
                sww = int(Sw[wl])
                g = gpool.tile([P, 1, SwMax * P + 16], bf16, tag="g")
                idxw = ipool.tile([128, SwMax * 8 + 8], i16, tag="idxw")
                icol0 = int(subbase[wl]) * 8
                nc.gpsimd.dma_start(out=idxw[:, :sww * 8],
                                    in_=t_idx[:, icol0:icol0 + sww * 8])
                gins = []
                with tc.tile_critical():
                    for (wl2, is_lo, off, L, icol_rel) in pl.gops:
                        if wl2 != wl:
                            continue
                        src = tab[:, 0:pl.split] if is_lo else tab[:, pl.split:NTAB]
                        goff = (0 if is_lo else S_lo * P) + off
                        q = gcount[0] % GQ
                        gg = nc.gpsimd.dma_gather(
                            out_ap=g[:, :, goff:goff + L],
                            in_ap=src,
                            idxs_ap=idxw[:, icol_rel:icol_rel + L // 16],
                            num_idxs=L, num_idxs_reg=L, elem_size=HID,
                            transpose=True,
                            sbuf_tokens_per_rank=128,
                            sbuf_free_dim_per_rank=256,
                            queue_num=q,
                        )
                        gg.then_inc(gsems[q], 16)
                        gins.append(gg.ins)
                        gcount[0] += 1
                        qcount[q] += 1
                return g, tuple(qcount), gins

            def wait_gathers_pe(g, cums, gins):
                wis = []
                with tc.tile_critical():
                    for q in range(GQ):
                        if cums[q]:
                            wi = nc.gpsimd.wait_ge(gsems[q], 16 * cums[q])
                            for gi_ins in gins:
                                add_dep_helper(wi.ins, gi_ins, sync=False,
                                               reason="gather wait ordering")
                            wis.append(wi.ins)
                    nc.gpsimd.memset(g[:, :, SwMax * P:], 0)
                return wis

            # ---------- embedding ----------
            # replicated bf16 table (bias folded via h0T's 1.0 row)
            eng_flip = [0]
            off = 0
            while off < NTAB:
                lw = min(512, NTAB - off)
                h0t = wp.tile([16, 512], bf16, tag="h0t")
                nc.sync.dma_start(out=h0t[:, :lw], in_=t_h0T[:, off:off + lw])
                for c0 in range(0, lw, 512):
                    cw = min(512, lw - c0)
                    ps = pz.tile([P, 512], f32, tag="pz")
                    for j in range(0, cw, 128):
                        nc.tensor.matmul(out=ps[:, j:j + 128],
                                         lhsT=h0t[:, c0 + j:c0 + j + 128],
                                         rhs=c_emb16[:], start=True, stop=True)
                    if eng_flip[0] % 2 == 0:
                        nc.vector.tensor_copy(out=tab[:, off + c0:off + c0 + cw],
                                              in_=ps[:, :cw])
                    else:
                        nc.scalar.copy(out=tab[:, off + c0:off + c0 + cw],
                                       in_=ps[:, :cw])
                    eng_flip[0] += 1
                off += lw
            # own-shard f32 feature-major hT
            off = 0
            while off < NS:
                lw = min(512, NS - off)
                h0o = wp.tile([16, 512], f32, tag="h0o", bufs=1)
                nc.sync.dma_start(out=h0o[:, :lw], in_=t_h0To[:, off:off + lw])
                for c0 in range(0, lw, 512):
                    cw = min(512, lw - c0)
                    ps = pe2.tile([P, 512], f32, tag="pe2")
                    nc.tensor.matmul(out=ps[:, :cw],
                                     lhsT=c_emb16_32[:],
                                     rhs=h0o[:, c0:c0 + cw],
                                     start=True, stop=True)
                    nc.scalar.copy(out=hT[:, off + c0:off + c0 + cw], in_=ps[:, :cw])
                off += lw

            # ---------- layers ----------
            for l in range(NL):
                def preload(wl):
                    sww = int(Sw[wl])
                    mems_t = mpool.tile([P, 2 * SwMax * P], f8 if MEMS_FP8 else bf16,
                    tag="mems", bufs=3 if MEMS_FP8 else 2)
                    mb = 2 * int(subbase[wl]) * P
                    nc.sync.dma_start(out=mems_t[:, :2 * sww * P],
                                      in_=t_mems[:, mb:mb + 2 * sww * P])
                    ra_t = rapool.tile([8, SwMax * P], bf16, tag="ra")
                    rb = int(subbase[wl]) * P
                    nc.gpsimd.dma_start(out=ra_t[:, :sww * P],
                                        in_=t_ra[:, rb:rb + sww * P])
                    # per-window row-side expansion weights: qw = h_win @ W1r
                    qps = pqs.tile([P, P], f32, tag="qps")
                    nc.tensor.matmul(out=qps[:],
                                     lhsT=hT[:, wl * P:(wl + 1) * P],
                                     rhs=c_w1r32[l][:],
                                     start=True, stop=True)
                    qw = wp.tile([P, P], bf16, tag="qw")
                    nc.scalar.copy(out=qw[:], in_=qps[:])
                    return mems_t, ra_t, qw

                blocks = []
                for wl in range(WPC):
                    nb = (int(Sw[wl]) + 3) // 4
                    for b in range(nb):
                        s0 = 4 * b
                        scnt = min(4, int(Sw[wl]) - s0)
                        blocks.append((wl, s0, scnt, b == 0, b == nb - 1))
                NBLK = len(blocks)
                pending_g = [issue_gathers(0), issue_gathers(1)]
                pend_loads = [preload(0)]
                win_state = {}
                pair_state = {}
                bt = {}

                def stage0(i):
                    wl, s0, scnt, first, last = blocks[i]
                    if first:
                        g, cums, gins = pending_g.pop(0)
                        mems_t, ra_t, qw = pend_loads.pop(0)
                        if wl + 1 < WPC:
                            pend_loads.append(preload(wl + 1))
                        wis = wait_gathers_pe(g, cums, gins)
                        if wl + 2 < WPC:
                            pending_g.append(issue_gathers(wl + 2))
                        win_state[wl] = (g, mems_t, ra_t, qw, wis)
                        if wl % 2 == 0:
                            pair_state[wl // 2] = (
                                pagg.tile([P, 256], f32, tag="pagg", name="p_agg"),
                                npl.tile([P, 256], bf16, tag="aggsb", name="agg_sb"),
                            )
                    g, mems_t, ra_t, qw, wis = win_state[wl]
                    W = scnt * P
                    p_z = pz.tile([P, 512], f32, tag="pz")
                    nc.tensor.matmul(out=p_z[:, :W], lhsT=qw[:],
                                     rhs=mems_t[:, s0 * P:s0 * P + W],
                                     start=True, stop=False)
                    nc.tensor.matmul(out=p_z[:, :W], lhsT=c_w1c[l][:],
                                     rhs=(tab[:, s0 * P:s0 * P + W]
                                          if ABL_NO_G else g[:, 0, s0 * P:s0 * P + W]),
                                     start=False, stop=False)
                    nc.tensor.matmul(out=p_z[:, :W], lhsT=c_w1ra[l][:],
                                     rhs=ra_t[:, s0 * P:s0 * P + W],
                                     start=False, stop=True)
                    bt[i] = {"p_z": p_z}

                def stage1(i):
                    # elu(z)+1 = relu(z) + min(exp(z), 1): exp/relu read PSUM
                    # directly on ACT, min/add are cheap bf16 DVE ops
                    wl, s0, scnt, first, last = blocks[i]
                    W = scnt * P
                    p_z = bt[i]["p_z"]
                    e1 = ew.tile([P, 512], bf16, tag="e1")
                    nc.scalar.activation(out=e1[:, :W], in_=p_z[:, :W], func=AF.Exp)
                    r1 = ew.tile([P, 512], bf16, tag="r1", bufs=1)
                    nc.scalar.activation(out=r1[:, :W], in_=p_z[:, :W], func=AF.Relu)
                    m1 = ew.tile([P, 512], bf16, tag="m1")
                    nc.vector.tensor_scalar(out=m1[:, :W], in0=e1[:, :W],
                                            scalar1=1.0, scalar2=None, op0=OP.min)
                    ef1 = ew.tile([P, 512], bf16, tag="ef1")
                    nc.vector.tensor_tensor(out=ef1[:, :W], in0=r1[:, :W],
                                            in1=m1[:, :W], op=OP.add)
                    bt[i]["ef1"] = ef1

                def stage2(i):
                    wl, s0, scnt, first, last = blocks[i]
                    ef1 = bt[i]["ef1"]
                    p_e2 = pe2.tile([P, 512], f32, tag="pe2")
                    for j in range(scnt):
                        sl_j = slice(j * P, (j + 1) * P)
                        nc.tensor.matmul(out=p_e2[:, sl_j], lhsT=ef1[:, sl_j],
                                         rhs=c_w2[l][:], start=True, stop=False)
                        nc.tensor.matmul(out=p_e2[:, sl_j], lhsT=c_ones[:],
                                         rhs=c_b2[l][:], start=False, stop=True)
                    bt[i]["p_e2"] = p_e2

                def stage3(i):
                    wl, s0, scnt, first, last = blocks[i]
                    W = scnt * P
                    p_e2 = bt[i]["p_e2"]
                    e2 = ew.tile([P, 512], bf16, tag="e2")
                    nc.scalar.activation(out=e2[:, :W], in_=p_e2[:, :W], func=AF.Exp)
                    r2 = ew.tile([P, 512], bf16, tag="r2", bufs=1)
                    nc.vector.tensor_scalar(out=r2[:, :W], in0=p_e2[:, :W],
                                            scalar1=0.0, scalar2=None, op0=OP.max)
                    m2 = ew.tile([P, 512], bf16, tag="m2")
                    nc.vector.tensor_scalar(out=m2[:, :W], in0=e2[:, :W],
                                            scalar1=1.0, scalar2=None, op0=OP.min)
                    ef2 = ew.tile([P, 512], bf16, tag="ef2")
                    nc.vector.tensor_tensor(out=ef2[:, :W], in0=r2[:, :W],
                                            in1=m2[:, :W], op=OP.add)
                    bt[i]["ef2"] = ef2

                def node_head(l, wl):
                    ph = wl % 2 if wl != WPC - 1 or wl % 2 == 1 else 0
                    wl0 = wl - (wl % 2)
                    NN = (wl - wl0 + 1) * P
                    nr = slice(wl0 * P, wl0 * P + NN)
                    agg_sb = pair_state[wl // 2][1]
                    hTb = wp.tile([P, 256], bf16, tag="hTb")
                    nc.vector.tensor_copy(out=hTb[:, :NN], in_=hT[:, nr])
                    p_nf = pnf.tile([P, 256], f32, tag="pnf")
                    nc.tensor.matmul(out=p_nf[:, :NN],
                                     lhsT=c_nw1ac[l][:],
                                     rhs=hTb[:, :NN],
                                     start=True, stop=False)
                    nc.tensor.matmul(out=p_nf[:, :NN],
                                     lhsT=c_nw1b[l][:],
                                     rhs=agg_sb[:, :NN],
                                     start=False, stop=False)
                    degc = wp.tile([1, 256], bf16, tag="degc", bufs=1)
                    nc.scalar.dma_start(out=degc[:, :NN], in_=t_deg[:, nr])
                    nc.tensor.matmul(out=p_nf[:, :NN],
                                     lhsT=c_nw1dg[l][:],
                                     rhs=degc[:, :NN],
                                     start=False, stop=True)
                    en = npl.tile([P, 256], f32, tag="en", bufs=1)
                    nc.scalar.activation(out=en[:, :NN], in_=p_nf[:, :NN],
                                         func=AF.Exp, bias=c_nb1[l][:])
                    mn = npl.tile([P, 256], f32, tag="mn", bufs=1)
                    nc.vector.tensor_scalar(out=mn[:, :NN], in0=en[:, :NN],
                                            scalar1=1.0, scalar2=None, op0=OP.min)
                    rn = npl.tile([P, 256], f32, tag="rn", bufs=1)
                    nc.vector.tensor_scalar(out=rn[:, :NN], in0=p_nf[:, :NN],
                                            scalar1=c_nb1[l][:], scalar2=0.0,
                                            op0=OP.add, op1=OP.max)
                    nf1 = npl.tile([P, 256], bf16, tag="nf1", bufs=1)
                    nc.vector.tensor_tensor(out=nf1[:, :NN], in0=rn[:, :NN],
                                            in1=mn[:, :NN], op=OP.add)
                    p_h = pnf.tile([P, 256], f32, tag="pnf")
                    nc.tensor.matmul(out=p_h[:, :NN],
                                     lhsT=c_nw2[l][:],
                                     rhs=nf1[:, :NN],
                                     start=True, stop=True)
                    nc.scalar.activation(out=hT[:, nr], in_=p_h[:, :NN],
                                         func=AF.Identity, bias=c_nb2[l][:])
                    if l == 0:
                        hb = wp.tile([P, 256], bf16, tag="hb")
                        for wi_ in range(NN // P):
                            tp = pz.tile([P, 512], f32, tag="pz")
                            nc.tensor.transpose(
                                out=tp[:, :P],
                                in_=hT[:, (wl0 + wi_) * P:(wl0 + wi_ + 1) * P],
                                identity=c_ident32[:])
                            nc.vector.tensor_copy(out=hb[:, wi_ * P:(wi_ + 1) * P],
                                                  in_=tp[:, :P])
                        nc.sync.dma_start(out=cc_in[:, nr], in_=hb[:, :NN])
                    if l == NL - 1:
                        lab = hp.tile([8, 256], bf16, tag="lab", bufs=1)
                        nc.scalar.dma_start(out=lab[:, :NN], in_=t_lab[:, nr])
                        epst = hp.tile([LAT, 256], f32, tag="epst", bufs=1)
                        nc.scalar.dma_start(out=epst[:, :NN], in_=t_eps[:, nr])
                        hTb2 = wp.tile([P, 256], bf16, tag="hTb2")
                        nc.vector.tensor_copy(out=hTb2[:, :NN], in_=hT[:, nr])
                        p_mu = pnf.tile([P, 256], f32, tag="pnf")
                        nc.tensor.matmul(out=p_mu[:LAT, :NN],
                                         lhsT=c_muw1[:],
                                         rhs=hTb2[:, :NN],
                                         start=True, stop=False)
                        nc.tensor.matmul(out=p_mu[:LAT, :NN],
                                         lhsT=c_muw2[:],
                                         rhs=lab[:, :NN],
                                         start=False, stop=True)
                        p_lv = pe2.tile([P, 512], f32, tag="pe2")
                        nc.tensor.matmul(out=p_lv[:LAT, :NN],
                                         lhsT=c_varw1[:],
                                         rhs=hTb2[:, :NN],
                                         start=True, stop=False)
                        nc.tensor.matmul(out=p_lv[:LAT, :NN],
                                         lhsT=c_varw2[:],
                                         rhs=lab[:, :NN],
                                         start=False, stop=True)
                        std = hp.tile([LAT, 256], f32, tag="std", bufs=1)
                        nc.scalar.activation(out=std[:, :NN], in_=p_lv[:LAT, :NN],
                                             func=AF.Exp, scale=0.5)
                        m1h = hp.tile([LAT, 256], f32, tag="m1h", bufs=1)
                        nc.vector.tensor_tensor(out=m1h[:, :NN], in0=epst[:, :NN],
                                                in1=std[:, :NN], op=OP.mult)
                        zc = hp.tile([LAT, 256], f32, tag="zc", bufs=1)
                        nc.vector.tensor_tensor(out=zc[:, :NN], in0=p_mu[:LAT, :NN],
                                                in1=m1h[:, :NN], op=OP.add)
                        nc.scalar.dma_start(out=t_z[:, nr], in_=zc[:, :NN])

                def stage4(i):
                    wl, s0, scnt, first, last = blocks[i]
                    _, mems_t, _, _, _ = win_state[wl]
                    sww = int(Sw[wl])
                    ph = wl % 2
                    p_agg, agg_sb = pair_state[wl // 2]
                    ef2 = bt[i]["ef1"] if ABL_NO_STAGE23 else bt[i]["ef2"]
                    for j in ([] if ABL_NO_SCATTER else range(scnt)):
                        s = s0 + j
                        nc.tensor.matmul(
                            out=p_agg[:, ph * P:(ph + 1) * P],
                            lhsT=ef2[:, j * P:(j + 1) * P],
                            rhs=mems_t[:, (sww + s) * P:(sww + s + 1) * P],
                            start=(s == 0),
                            stop=(s == sww - 1))
                    if last and not ABL_NO_SCATTER:
                        nc.vector.tensor_copy(out=agg_sb[:, ph * P:(ph + 1) * P],
                                              in_=p_agg[:, ph * P:(ph + 1) * P])
                        if (ph == 1 or wl == WPC - 1) and not EDGE_ONLY:
                            node_head(l, wl)
                        del win_state[wl]
                    del bt[i]

                for i in range(NBLK + 4):
                    if i < NBLK:
                        stage0(i)
                    if 1 <= i and i - 1 < NBLK:
                        stage1(i - 1)
                    if not ABL_NO_STAGE23:
                        if 2 <= i and i - 2 < NBLK:
                            stage2(i - 2)
                        if 3 <= i and i - 3 < NBLK:
                            stage3(i - 3)
                    if 4 <= i and i - 4 < NBLK:
                        stage4(i - 4)

                # ---------- layer boundary: re-replicate h ----------
                if l == 0 and not EDGE_ONLY:
                    if not no_collective:
                        nc.gpsimd.collective_compute(
                            "AllGather", OP.bypass,
                            replica_groups=[list(range(N_CORES))],
                            ins=[cc_in[:]], outs=[cc_out[:]],
                        )
                    nc.sync.dma_start(
                        out=tab[:, :NTAB].rearrange("p (k c) -> p k c", k=N_CORES),
                        in_=cc_out.rearrange("k p c -> p k c"),
                    )
    nc.compile()
    return nc


def _in_maps(pl):
    maps = []
    for k in range(N_CORES):
        m = {
            "idx": pl.idx[k], "ra": pl.ra[k], "mems": pl.mems[k], "deg": pl.deg[k],
            "h0T": pl.h0T, "h0T_own": pl.h0T_own[k], "labelT": pl.labelT[k],
            "epsT": pl.epsT[k],
            "emb16": pl.emb16, "emb16_32": pl.emb16_32,
            "muw1": pl.muw1, "muw2": pl.muw2, "varw1": pl.varw1, "varw2": pl.varw2,
            "ident32": pl.ident32, "ones_row": pl.ones_row,
        }
        for l in range(NL):
            m[f"w1r32{l}"] = pl.w1r32[l]
            m[f"w1c{l}"] = pl.w1c[l]
            m[f"w1ra{l}"] = pl.w1ra[l]
            m[f"w2{l}"] = pl.w2[l]
            m[f"b2adj{l}"] = pl.b2adj[l]
            m[f"nw1ac{l}"] = pl.nw1ac[l]
            m[f"nw1b{l}"] = pl.nw1b[l]
            m[f"nw1deg{l}"] = pl.nw1deg[l]
            m[f"nb1{l}"] = pl.nb1[l]
            m[f"nw2{l}"] = pl.nw2[l]
            m[f"nb2adj{l}"] = pl.nb2adj[l]
        maps.append(m)
    return maps


def _prep_and_build(inputs):
    pl = _host_prep(inputs)
    key = (pl.TS, tuple(pl.SL), tuple(pl.SH), pl.NTAB, pl.split)
    if key not in _compile_cache:
        _compile_cache[key] = build_nc(pl)
    return pl, _compile_cache[key]


def kernel(**inputs):
    pl, nc = _prep_and_build(inputs)
    maps = _in_maps(pl)
    res = run_bass_kernel_spmd(nc, maps, list(range(N_CORES)))
    z = np.concatenate([res.results[k]["z"] for k in range(N_CORES)], axis=1)
    return np.ascontiguousarray(z[:, :pl.N].T).astype(np.float32)
